# revision 29
# baseline (speedup 1.0000x reference)
"""GCN layer (message passing + linear + ReLU) on 8 Trainium2 NeuronCores.

out = relu(((scatter_add(h[src] -> dst) + x) * dis) @ W.T),
h = x * dis,  dis = rsqrt(deg + 1),  deg = in-degree via dst counts.

Strategy (SPMD, one program on 8 cores):
  - Nodes sharded contiguously: core c owns rows [c*6250, (c+1)*6250).
  - Host partitions edges by dst owner and sorts by dst (index-only work);
    degree reaches the device as CSR rowptr slices, so deg = rowptr diff
    and dis = 1/sqrt(deg+1) are computed on device in f32.
  - No h table: each edge's h[src] = x[src]*dis[src] is obtained by
    gathering x[src] (bf16, 256B rows) straight from a replicated bf16
    copy of x, with dis[src] folded into the one-hot scatter matrix S
    (S values are host-precomputed per-edge scales -- graph metadata).
  - Scatter-add on-chip: edges sorted by dst fall into windows of 128
    owned nodes; per 128-edge chunk S[e, slot] = dis[src]*(slot==dst-base)
    is built on DVE and the PE accumulates gt.T @ S = agg.T [feat, slot]
    into the window's PSUM bank.
  - S is built in fixed groups of 8 chunks, stored column-major
    (S[p, col*8 + chunk]) so every DVE operand has a packed last dim and
    the 16-bit 2x DVE mode engages; the matmul rhs reads its chunk with a
    stride-8 AP (free for PE: partition dim is the parallel dim).
  - int16 gather indices can't span 50k rows, so each window's edges are
    processed against table halves x[:32768] / x[32768:] (pass lo/hi).
  - Gather calls round-robin over 4 SWDGE queues (each queue is served
    by a different pair of Q7 cores, so descriptor generation overlaps).
  - gidx/xst uploads are split so the first gather/finalize only waits
    for a small first segment.
  - Finalize per window, fused right after its last matmul: att =
    (psum + xT) in bf16, po = att.T @ W.T via PE (no transpose needed:
    agg is feature-major), out = relu(po * dis_dst) via ACT per-partition
    scale, then DMA out.
Chunk counts per (pass, window) are maxed over cores so the single SPMD
program fits every core; shorter cores pad with slot=255 / idx=0 chunks.
"""
import numpy as np
import ml_dtypes

from concourse import bacc, bass, mybir, tile
from concourse.bass_utils import run_bass_kernel_spmd

F32 = mybir.dt.float32
BF16 = mybir.dt.bfloat16
I32 = mybir.dt.int32
I16 = mybir.dt.int16
AF = mybir.ActivationFunctionType
OP = mybir.AluOpType

N = 50000
E = 600000
D = 128
C = 8                      # cores
NPC = N // C               # 6250 nodes per core
WPC = (NPC + 127) // 128   # 49 windows per core
NPAD = WPC * 128           # 6272 padded shard rows
NT_G = (N + 127) // 128    # 391 global node tiles
NROWS = NT_G * 128         # 50048 padded table rows
SPLIT = 32768              # src table split for int16 gather indices
PASS_BOUNDS = [(0, SPLIT), (SPLIT, N)]
GB = 8                     # max chunks per dma_gather call (1024 idxs; >1024
                           # descriptors per SWDGE call crashes the device)
SG = 8                     # chunks per S-group build
NQ = 4                     # SWDGE queues used round-robin
GIDX_SPLITS = [0, 4, 16, 27, 38, WPC]  # gidx upload part boundaries (windows)
XST_PARTS = 7              # xst upload split (7 windows each)


def _chunk_layout(K):
    """Global chunk index base per (pass, window), window-major interleaved."""
    K = np.asarray(K)
    cbase = np.zeros((2, WPC), np.int64)
    cb = 0
    for w in range(WPC):
        for p in range(2):
            cbase[p, w] = cb
            cb += K[p, w]
    return cbase, int(cb)


# ---------------------------------------------------------------- host prep
def host_prep(edge_index):
    src = np.asarray(edge_index[0], dtype=np.int64)
    dst = np.asarray(edge_index[1], dtype=np.int64)
    order = np.argsort(dst, kind="stable")
    ss_all = src[order]
    dd_all = dst[order]
    counts = np.bincount(dst, minlength=N)
    rowptr = np.zeros(N + 1, np.int64)
    rowptr[1:] = np.cumsum(counts)
    dis = 1.0 / np.sqrt(counts.astype(np.float64) + 1.0)  # rsqrt(deg+1)

    per_core = []
    need = np.zeros((C, 2, WPC), np.int64)
    for c in range(C):
        e0, e1 = rowptr[c * NPC], rowptr[(c + 1) * NPC]
        ss, dd = ss_all[e0:e1], dd_all[e0:e1]
        per_core.append((ss, dd))
        for p, (lo, hi) in enumerate(PASS_BOUNDS):
            m = (ss >= lo) & (ss < hi)
            w = (dd[m] - c * NPC) // 128
            need[c, p] = np.bincount(w, minlength=WPC)
    K = np.ceil(need.max(axis=0) / 128).astype(np.int64)  # [2, WPC]

    cbase, TC = _chunk_layout(K)
    TC8 = ((TC + SG - 1) // SG) * SG

    cores = []
    for c in range(C):
        ss, dd = per_core[c]
        g = np.full(TC * 128, -1, np.int64)
        s = np.full(TC8 * 128, 255, np.int64)
        sc = np.zeros(TC8 * 128, np.float64)
        for p, (lo, hi) in enumerate(PASS_BOUNDS):
            m = (ss >= lo) & (ss < hi)
            sg = ss[m]                       # global src id
            sp = sg - lo                     # index into table half
            dloc = dd[m] - c * NPC
            w = dloc // 128
            cnt = np.bincount(w, minlength=WPC)
            ofs = np.zeros(WPC, np.int64)
            ofs[1:] = np.cumsum(cnt[:-1])
            pos = cbase[p, w] * 128 + (np.arange(len(sp)) - ofs[w])
            g[pos] = sp
            s[pos] = dloc - w * 128
            sc[pos] = dis[sg]
        d = {}
        # pad gather slots stay -1: they are trailing within each (pass,
        # window) block, so the SWDGE q7 kernel trims them and skips their
        # descriptors entirely (negative idx = ignored-at-end semantics).
        # The decode-side ring accounting sizes from num_idxs_reg, so each
        # call passes its per-core valid count via a Pool register (nidx).
        nidx = []
        for w in range(WPC):
            for p in range(2):
                cnt = int(need[c, p, w])
                Kw = int(K[p, w])
                done = 0
                while done < Kw:
                    nch = min(8, Kw - done)
                    nidx.append(max(0, min(cnt - done * 128, nch * 128)))
                    done += nch
        d["nidx"] = np.asarray(nidx, np.int32)[None, :].copy()
        # gather idx layout [128, TC*8]: stream pos j at [j%16, j//16],
        # replicated across the 8 groups of 16 partitions.
        d["gidx"] = np.tile(g.reshape(-1, 16).T.astype(np.int16), (8, 1)).copy()
        # slot / scale layout [128, TC8]: stream pos j at [j%128, j//128]
        d["slots"] = s.reshape(-1, 128).T.astype(np.int16).copy()
        d["scales"] = sc.reshape(-1, 128).T.astype(ml_dtypes.bfloat16).copy()
        n0 = c * NPC
        rpv = np.full(NPAD + 1, rowptr[min((c + 1) * NPC, N)], np.int64)
        rpv[: NPC + 1] = rowptr[n0 : n0 + NPC + 1]
        d["rp0s"] = rpv[:NPAD].reshape(WPC, 128).T.astype(np.int32).copy()
        d["rp1s"] = rpv[1 : NPAD + 1].reshape(WPC, 128).T.astype(np.int32).copy()
        cores.append(d)
    return dict(K=K, cbase=cbase, TC=TC, TC8=TC8, cores=cores)


def _gidx_parts(K, cbase, TC):
    """Split windows into ranges per GIDX_SPLITS; return per-part
    (w0, w1, chunk_start, chunk_end) so gather calls address their tile."""
    parts = []
    for i in range(len(GIDX_SPLITS) - 1):
        w0, w1 = GIDX_SPLITS[i], GIDX_SPLITS[i + 1]
        c0 = int(cbase[0, w0])
        c1 = int(cbase[0, w1]) if w1 < WPC else TC
        parts.append((w0, w1, c0, c1))
    return parts


# ---------------------------------------------------------------- program
def build_program(K):
    K = np.asarray(K)
    cbase, TC = _chunk_layout(K)
    TC8 = ((TC + SG - 1) // SG) * SG
    NSG = TC // SG + (1 if TC % SG else 0)  # S groups actually consumed

    nc = bacc.Bacc(
        None, target_bir_lowering=False, debug=False, num_swdge_queues=NQ
    )

    x_p = nc.dram_tensor("xb", [NROWS, D], BF16, kind="ExternalInput")
    xst_p = nc.dram_tensor("xst", [D, NPAD], BF16, kind="ExternalInput")
    wt_p = nc.dram_tensor("wt", [D, D], BF16, kind="ExternalInput")
    iota8_p = nc.dram_tensor("iota8", [128, 128 * SG], BF16, kind="ExternalInput")
    rp0s_p = nc.dram_tensor("rp0s", [128, WPC], I32, kind="ExternalInput")
    rp1s_p = nc.dram_tensor("rp1s", [128, WPC], I32, kind="ExternalInput")
    gidx_p = nc.dram_tensor("gidx", [128, TC * 8], I16, kind="ExternalInput")
    slots_p = nc.dram_tensor("slots", [128, TC8], I16, kind="ExternalInput")
    scales_p = nc.dram_tensor("scales", [128, TC8], BF16, kind="ExternalInput")
    ncalls = 0
    for w in range(WPC):
        for p in range(2):
            ncalls += -(-int(K[p, w]) // GB)
    nidx_p = nc.dram_tensor("nidx", [1, ncalls], I32, kind="ExternalInput")
    out_p = nc.dram_tensor("out", [NPAD, D], BF16, kind="ExternalOutput")

    gparts = _gidx_parts(K, cbase, TC)

    with tile.TileContext(nc) as tc:
        with (
            tc.tile_pool(name="const", bufs=1) as cpool,
            tc.tile_pool(name="gather", bufs=16) as gpool,
            tc.tile_pool(name="sel", bufs=8) as spool,
            tc.tile_pool(name="fin", bufs=3) as fpool,
            tc.tile_pool(name="psA", bufs=4, space="PSUM") as psA,
            tc.tile_pool(name="psO", bufs=2, space="PSUM") as psO,
        ):
            # --- uploads; gidx parts on the sync queue (first part small so
            # gathers start early), metadata on the scalar queue, xst on the
            # vector queue -- three independent HWDGE rings.
            gidx_sb = {}
            part_of_w = {}
            for i, (w0, w1, c0, c1) in enumerate(gparts):
                gt_ = cpool.tile([128, (c1 - c0) * 8], I16, tag=f"gidx{i}")
                gidx_sb[i] = (gt_, c0)
                for w in range(w0, w1):
                    part_of_w[w] = i
            nc.sync.dma_start(
                gidx_sb[0][0][:], gidx_p[:, gparts[0][2] * 8 : gparts[0][3] * 8]
            )
            nidx_sb = cpool.tile([1, ncalls], I32, tag="nidx")
            nc.scalar.dma_start(nidx_sb[:], nidx_p[:])
            si = cpool.tile([128, TC8], I16, tag="si")
            nc.scalar.dma_start(si[:], slots_p[:])
            scf = cpool.tile([128, TC8], BF16, tag="scf")
            nc.scalar.dma_start(scf[:], scales_p[:])
            iota8_sb = cpool.tile([128, 128 * SG], BF16, tag="iota8")
            nc.scalar.dma_start(iota8_sb[:], iota8_p[:])
            wt_sb = cpool.tile([128, 128], BF16, tag="wt")
            nc.scalar.dma_start(wt_sb[:], wt_p[:])
            r0i = cpool.tile([128, WPC], I32, tag="r0i")
            nc.scalar.dma_start(r0i[:], rp0s_p[:])
            r1i = cpool.tile([128, WPC], I32, tag="r1i")
            nc.scalar.dma_start(r1i[:], rp1s_p[:])
            nidx_r = nc.gpsimd.alloc_register("nidx_r")

            sf = cpool.tile([128, TC8], BF16, tag="sf")
            nc.vector.tensor_copy(sf[:], si[:])

            # dis_dst = 1/sqrt(deg+1) from rowptr diffs, [128, WPC] f32
            r0f = cpool.tile([128, WPC], F32, tag="r0f")
            nc.vector.tensor_copy(r0f[:], r0i[:])
            r1f = cpool.tile([128, WPC], F32, tag="r1f")
            nc.vector.tensor_copy(r1f[:], r1i[:])
            dg = cpool.tile([128, WPC], F32, tag="dg")
            nc.vector.tensor_tensor(out=dg[:], in0=r1f[:], in1=r0f[:], op=OP.subtract)
            nc.vector.tensor_scalar_add(out=dg[:], in0=dg[:], scalar1=1.0)
            rc = cpool.tile([128, WPC], F32, tag="rc")
            nc.vector.reciprocal(rc[:], dg[:])
            dis_s = cpool.tile([128, WPC], F32, tag="dis")
            nc.scalar.activation(dis_s[:], rc[:], AF.Sqrt)

            # remaining gidx parts (sync queue), xst parts (vector queue)
            for i in range(1, len(gparts)):
                t, c0 = gidx_sb[i]
                nc.sync.dma_start(t[:], gidx_p[:, c0 * 8 : gparts[i][3] * 8])
            xst_sb = cpool.tile([128, NPAD], BF16, tag="xst")
            xw = (WPC + XST_PARTS - 1) // XST_PARTS  # windows per xst part
            for i in range(XST_PARTS):
                a, b = i * xw * 128, min((i + 1) * xw * 128, NPAD)
                nc.scalar.dma_start(xst_sb[:, a:b], xst_p[:, a:b])

            tables = [x_p[0:SPLIT, :], x_p[SPLIT:NROWS, :]]
            out_v = out_p[:].rearrange("(u p) d -> p u d", p=128)

            # zero every gather buffer once: trimmed pad lanes leave stale
            # SBUF in gt rows, and uninitialized bf16 can be NaN/Inf --
            # 0 * NaN would poison the PSUM accumulation.
            for _ in range(16):
                gz = gpool.tile([128, GB * 128], BF16, tag="gt")
                nc.vector.memset(gz[:], 0.0)

            # --- S group builder: chunk-major S[p, k*128 + c] built per
            # SG-chunk group with two plain DVE passes (eq then scale).
            # Chunk-major keeps the matmul rhs contiguous; the broadcast
            # operands run at 1x DVE rate but per-group ops amortize well.
            sgroups = {}

            def build_sgroup(gb):
                Sw = spool.tile([128, 128 * SG], BF16, tag="S")
                sw = Sw[:]
                o = gb * SG
                dims = [sw.ap[0], [128, SG], [1, 128]]  # (k, c) iteration
                outap = bass.AP(sw.tensor, sw.offset, dims)
                in0 = bass.AP(sf.tensor, sf.offset + o, [sf.ap[0], [1, SG], [0, 128]])
                ii = iota8_sb[:]
                in1 = bass.AP(ii.tensor, ii.offset, [ii.ap[0], [128, SG], [1, 128]])
                nc.vector.tensor_tensor(out=outap, in0=in0, in1=in1, op=OP.is_equal)
                in2 = bass.AP(scf.tensor, scf.offset + o, [scf.ap[0], [1, SG], [0, 128]])
                nc.vector.tensor_tensor(out=outap, in0=outap, in1=in2, op=OP.mult)
                sgroups[gb] = Sw
                return Sw

            qrr = 0
            for w in range(WPC):
                nmm_w = int(K[0, w] + K[1, w])
                mm_w = 0
                ps = psA.tile([128, 128], F32, tag="pacc")
                for p in range(2):
                    Kw = int(K[p, w])
                    c0 = int(cbase[p, w])
                    done = 0
                    while done < Kw:
                        nch = min(GB, Kw - done)
                        cc = c0 + done
                        pi = part_of_w[w]
                        ptile, pbase = gidx_sb[pi]
                        lofs = (cc - pbase) * 8
                        gt = gpool.tile([128, GB * 128], BF16, tag="gt")
                        gv = gt[:, : nch * 128].rearrange(
                            "p (b e) -> p b e", e=128
                        )
                        nc.gpsimd.reg_load(nidx_r, nidx_sb[0:1, qrr : qrr + 1])
                        nc.gpsimd.dma_gather(
                            gv,
                            tables[p],
                            ptile[:, lofs : lofs + nch * 8],
                            nch * 128,
                            nidx_r,
                            D,
                            queue_num=qrr % NQ,
                        )
                        qrr += 1
                        for k in range(nch):
                            g = cc + k
                            gb, kk = divmod(g, SG)
                            Sw = sgroups.get(gb)
                            if Sw is None:
                                Sw = build_sgroup(gb)
                            nc.tensor.matmul(
                                ps[:],
                                lhsT=gt[:, k * 128 : (k + 1) * 128],
                                rhs=Sw[:, kk * 128 : (kk + 1) * 128],
                                start=(mm_w == 0),
                                stop=(mm_w == nmm_w - 1),
                            )
                            mm_w += 1
                        done += nch

                # --- finalize window w
                wsl = slice(w * 128, (w + 1) * 128)
                att = fpool.tile([128, 128], BF16, tag="att")
                if nmm_w:
                    nc.vector.tensor_tensor(
                        out=att[:], in0=ps[:], in1=xst_sb[:, wsl], op=OP.add
                    )
                else:
                    nc.vector.tensor_copy(att[:], xst_sb[:, wsl])
                po = psO.tile([128, 128], F32, tag="po")
                nc.tensor.matmul(
                    po[:], lhsT=att[:], rhs=wt_sb[:], start=True, stop=True
                )
                ot = fpool.tile([128, 128], BF16, tag="ot")
                nc.scalar.activation(
                    ot[:], po[:], AF.Relu, scale=dis_s[:, w : w + 1]
                )
                nc.sync.dma_start(out_v[:, w, :], ot[:])

    nc.compile()
    return nc


# ---------------------------------------------------------------- runner
_CACHE = {}


def _get_program(K):
    key = K.tobytes()
    if key not in _CACHE:
        _CACHE[key] = build_program(K)
    return _CACHE[key]


def make_in_maps(x, W, prep):
    x = np.asarray(x, np.float32)
    xb = np.zeros((NROWS, D), ml_dtypes.bfloat16)
    xb[:N] = x.astype(ml_dtypes.bfloat16)
    Wt = np.ascontiguousarray(np.asarray(W, np.float32).T).astype(
        ml_dtypes.bfloat16
    )
    # iota8[p, k*128 + c] = c  (chunk-major)
    iota8 = np.tile(
        np.tile(np.arange(128, dtype=np.float32), SG)[None, :], (128, 1)
    ).astype(ml_dtypes.bfloat16)
    in_maps = []
    for c in range(C):
        cd = prep["cores"][c]
        xst = np.zeros((D, NPAD), ml_dtypes.bfloat16)
        xst[:, :NPC] = x[c * NPC : (c + 1) * NPC].T.astype(ml_dtypes.bfloat16)
        in_maps.append(
            {
                "xb": xb,
                "xst": xst,
                "wt": Wt,
                "iota8": iota8,
                "rp0s": cd["rp0s"],
                "rp1s": cd["rp1s"],
                "gidx": cd["gidx"],
                "slots": cd["slots"],
                "scales": cd["scales"],
                "nidx": cd["nidx"],
            }
        )
    return in_maps


def run_spmd(x, edge_index, W, trace=False, **spmd_kwargs):
    prep = host_prep(edge_index)
    nc = _get_program(prep["K"])
    in_maps = make_in_maps(x, W, prep)
    res = run_bass_kernel_spmd(nc, in_maps, list(range(C)), trace=trace, **spmd_kwargs)
    out = np.concatenate(
        [np.asarray(res.results[c]["out"][:NPC], np.float32) for c in range(C)],
        axis=0,
    )
    return out, res


def kernel(x, edge_index, N=None, W=None, **_):
    out, _res = run_spmd(np.asarray(x), np.asarray(edge_index), np.asarray(W))
    return out


# revision 32
# speedup vs baseline: 1.0109x; 1.0109x over previous
"""GCN layer (message passing + linear + ReLU) on 8 Trainium2 NeuronCores.

out = relu(((scatter_add(h[src] -> dst) + x) * dis) @ W.T),
h = x * dis,  dis = rsqrt(deg + 1),  deg = in-degree via dst counts.

Strategy (SPMD, one program on 8 cores):
  - Nodes sharded contiguously: core c owns rows [c*6250, (c+1)*6250).
  - Host partitions edges by dst owner and sorts by dst (index-only work);
    degree reaches the device as CSR rowptr slices, so deg = rowptr diff
    and dis = 1/sqrt(deg+1) are computed on device in f32.
  - No h table: each edge's h[src] = x[src]*dis[src] is obtained by
    gathering x[src] (bf16, 256B rows) straight from a replicated bf16
    copy of x, with dis[src] folded into the one-hot scatter matrix S
    (S values are host-precomputed per-edge scales -- graph metadata).
  - Scatter-add on-chip: edges sorted by dst fall into windows of 128
    owned nodes; per 128-edge chunk S[e, slot] = dis[src]*(slot==dst-base)
    is built on DVE and the PE accumulates gt.T @ S = agg.T [feat, slot]
    into the window's PSUM bank.
  - S is built in fixed groups of 8 chunks, stored column-major
    (S[p, col*8 + chunk]) so every DVE operand has a packed last dim and
    the 16-bit 2x DVE mode engages; the matmul rhs reads its chunk with a
    stride-8 AP (free for PE: partition dim is the parallel dim).
  - int16 gather indices can't span 50k rows, so each window's edges are
    processed against table halves x[:32768] / x[32768:] (pass lo/hi).
  - Gather calls round-robin over 4 SWDGE queues (each queue is served
    by a different pair of Q7 cores, so descriptor generation overlaps).
  - gidx/xst uploads are split so the first gather/finalize only waits
    for a small first segment.
  - Finalize per window, fused right after its last matmul: att =
    (psum + xT) in bf16, po = att.T @ W.T via PE (no transpose needed:
    agg is feature-major), out = relu(po * dis_dst) via ACT per-partition
    scale, then DMA out.
Chunk counts per (pass, window) are maxed over cores so the single SPMD
program fits every core; shorter cores pad with slot=255 / idx=0 chunks.
"""
import numpy as np
import ml_dtypes

from concourse import bacc, bass, mybir, tile
from concourse.bass_utils import run_bass_kernel_spmd

F32 = mybir.dt.float32
BF16 = mybir.dt.bfloat16
I32 = mybir.dt.int32
I16 = mybir.dt.int16
AF = mybir.ActivationFunctionType
OP = mybir.AluOpType

N = 50000
E = 600000
D = 128
C = 8                      # cores
NPC = N // C               # 6250 nodes per core
WPC = (NPC + 127) // 128   # 49 windows per core
NPAD = WPC * 128           # 6272 padded shard rows
NT_G = (N + 127) // 128    # 391 global node tiles
NROWS = NT_G * 128         # 50048 padded table rows
SPLIT = 32768              # src table split for int16 gather indices
PASS_BOUNDS = [(0, SPLIT), (SPLIT, N)]
GB = 8                     # max chunks per dma_gather call (1024 idxs; >1024
                           # descriptors per SWDGE call crashes the device)
SG = 8                     # chunks per S-group build
NQ = 4                     # SWDGE queues used round-robin
GIDX_SPLITS = [0, 4, 16, 27, 38, WPC]  # gidx upload part boundaries (windows)
XST_PARTS = 7              # xst upload split (7 windows each)


def _chunk_layout(K):
    """Global chunk index base per (pass, window), window-major interleaved."""
    K = np.asarray(K)
    cbase = np.zeros((2, WPC), np.int64)
    cb = 0
    for w in range(WPC):
        for p in range(2):
            cbase[p, w] = cb
            cb += K[p, w]
    return cbase, int(cb)


# ---------------------------------------------------------------- host prep
def host_prep(edge_index):
    src = np.asarray(edge_index[0], dtype=np.int64)
    dst = np.asarray(edge_index[1], dtype=np.int64)
    order = np.argsort(dst, kind="stable")
    ss_all = src[order]
    dd_all = dst[order]
    counts = np.bincount(dst, minlength=N)
    rowptr = np.zeros(N + 1, np.int64)
    rowptr[1:] = np.cumsum(counts)
    dis = 1.0 / np.sqrt(counts.astype(np.float64) + 1.0)  # rsqrt(deg+1)

    per_core = []
    need = np.zeros((C, 2, WPC), np.int64)
    for c in range(C):
        e0, e1 = rowptr[c * NPC], rowptr[(c + 1) * NPC]
        ss, dd = ss_all[e0:e1], dd_all[e0:e1]
        per_core.append((ss, dd))
        for p, (lo, hi) in enumerate(PASS_BOUNDS):
            m = (ss >= lo) & (ss < hi)
            w = (dd[m] - c * NPC) // 128
            need[c, p] = np.bincount(w, minlength=WPC)
    K = np.ceil(need.max(axis=0) / 128).astype(np.int64)  # [2, WPC]

    cbase, TC = _chunk_layout(K)
    TC8 = ((TC + SG - 1) // SG) * SG

    cores = []
    for c in range(C):
        ss, dd = per_core[c]
        g = np.full(TC * 128, -1, np.int64)
        s = np.full(TC8 * 128, 255, np.int64)
        sc = np.zeros(TC8 * 128, np.float64)
        for p, (lo, hi) in enumerate(PASS_BOUNDS):
            m = (ss >= lo) & (ss < hi)
            sg = ss[m]                       # global src id
            sp = sg - lo                     # index into table half
            dloc = dd[m] - c * NPC
            w = dloc // 128
            cnt = np.bincount(w, minlength=WPC)
            ofs = np.zeros(WPC, np.int64)
            ofs[1:] = np.cumsum(cnt[:-1])
            pos = cbase[p, w] * 128 + (np.arange(len(sp)) - ofs[w])
            g[pos] = sp
            s[pos] = dloc - w * 128
            sc[pos] = dis[sg]
        d = {}
        # pad gather slots stay -1: they are trailing within each (pass,
        # window) block, so the SWDGE q7 kernel trims them and skips their
        # descriptors entirely (negative idx = ignored-at-end semantics).
        # The decode-side ring accounting sizes from num_idxs_reg, so each
        # call passes its per-core valid count via a Pool register (nidx).
        nidx = []
        for w in range(WPC):
            for p in range(2):
                cnt = int(need[c, p, w])
                Kw = int(K[p, w])
                done = 0
                while done < Kw:
                    nch = min(8, Kw - done)
                    nidx.append(max(0, min(cnt - done * 128, nch * 128)))
                    done += nch
        d["nidx"] = np.asarray(nidx, np.int32)[None, :].copy()
        # gather idx layout [128, TC*8]: stream pos j at [j%16, j//16],
        # replicated across the 8 groups of 16 partitions.
        d["gidx"] = np.tile(g.reshape(-1, 16).T.astype(np.int16), (8, 1)).copy()
        # slot / scale layout [128, TC8]: stream pos j at [j%128, j//128]
        d["slots"] = s.reshape(-1, 128).T.astype(np.int16).copy()
        d["scales"] = sc.reshape(-1, 128).T.astype(ml_dtypes.bfloat16).copy()
        n0 = c * NPC
        rpv = np.full(NPAD + 1, rowptr[min((c + 1) * NPC, N)], np.int64)
        rpv[: NPC + 1] = rowptr[n0 : n0 + NPC + 1]
        d["rp0s"] = rpv[:NPAD].reshape(WPC, 128).T.astype(np.int32).copy()
        d["rp1s"] = rpv[1 : NPAD + 1].reshape(WPC, 128).T.astype(np.int32).copy()
        cores.append(d)
    return dict(K=K, cbase=cbase, TC=TC, TC8=TC8, cores=cores)


def _gidx_parts(K, cbase, TC):
    """Split windows into ranges per GIDX_SPLITS; return per-part
    (w0, w1, chunk_start, chunk_end) so gather calls address their tile."""
    parts = []
    for i in range(len(GIDX_SPLITS) - 1):
        w0, w1 = GIDX_SPLITS[i], GIDX_SPLITS[i + 1]
        c0 = int(cbase[0, w0])
        c1 = int(cbase[0, w1]) if w1 < WPC else TC
        parts.append((w0, w1, c0, c1))
    return parts


# ---------------------------------------------------------------- program
def build_program(K):
    K = np.asarray(K)
    cbase, TC = _chunk_layout(K)
    TC8 = ((TC + SG - 1) // SG) * SG
    NSG = TC // SG + (1 if TC % SG else 0)  # S groups actually consumed

    nc = bacc.Bacc(
        None, target_bir_lowering=False, debug=False, num_swdge_queues=NQ
    )

    x_p = nc.dram_tensor("xb", [NROWS, D], BF16, kind="ExternalInput")
    xst_p = nc.dram_tensor("xst", [D, NPAD], BF16, kind="ExternalInput")
    wt_p = nc.dram_tensor("wt", [D, D], BF16, kind="ExternalInput")
    iota8_p = nc.dram_tensor("iota8", [128, 128 * SG], BF16, kind="ExternalInput")
    rp0s_p = nc.dram_tensor("rp0s", [128, WPC], I32, kind="ExternalInput")
    rp1s_p = nc.dram_tensor("rp1s", [128, WPC], I32, kind="ExternalInput")
    gidx_p = nc.dram_tensor("gidx", [128, TC * 8], I16, kind="ExternalInput")
    slots_p = nc.dram_tensor("slots", [128, TC8], I16, kind="ExternalInput")
    scales_p = nc.dram_tensor("scales", [128, TC8], BF16, kind="ExternalInput")
    ncalls = 0
    for w in range(WPC):
        for p in range(2):
            ncalls += -(-int(K[p, w]) // GB)
    nidx_p = nc.dram_tensor("nidx", [1, ncalls], I32, kind="ExternalInput")
    out_p = nc.dram_tensor("out", [NPAD, D], BF16, kind="ExternalOutput")

    gparts = _gidx_parts(K, cbase, TC)

    with tile.TileContext(nc) as tc:
        with (
            tc.tile_pool(name="const", bufs=1) as cpool,
            tc.tile_pool(name="gather", bufs=16) as gpool,
            tc.tile_pool(name="sel", bufs=8) as spool,
            tc.tile_pool(name="fin", bufs=3) as fpool,
            tc.tile_pool(name="psA", bufs=4, space="PSUM") as psA,
            tc.tile_pool(name="psO", bufs=2, space="PSUM") as psO,
        ):
            # --- uploads; gidx parts on the sync queue (first part small so
            # gathers start early), metadata on the scalar queue, xst on the
            # vector queue -- three independent HWDGE rings.
            gidx_sb = {}
            part_of_w = {}
            for i, (w0, w1, c0, c1) in enumerate(gparts):
                gt_ = cpool.tile([128, (c1 - c0) * 8], I16, tag=f"gidx{i}")
                gidx_sb[i] = (gt_, c0)
                for w in range(w0, w1):
                    part_of_w[w] = i
            nc.sync.dma_start(
                gidx_sb[0][0][:], gidx_p[:, gparts[0][2] * 8 : gparts[0][3] * 8]
            )
            nidx_sb = cpool.tile([1, ncalls], I32, tag="nidx")
            nc.scalar.dma_start(nidx_sb[:], nidx_p[:])
            r0i = cpool.tile([128, WPC], I32, tag="r0i")
            nc.scalar.dma_start(r0i[:], rp0s_p[:])
            r1i = cpool.tile([128, WPC], I32, tag="r1i")
            nc.scalar.dma_start(r1i[:], rp1s_p[:])
            si = cpool.tile([128, TC8], I16, tag="si")
            nc.scalar.dma_start(si[:], slots_p[:])
            scf = cpool.tile([128, TC8], BF16, tag="scf")
            nc.scalar.dma_start(scf[:], scales_p[:])
            iota8_sb = cpool.tile([128, 128 * SG], BF16, tag="iota8")
            nc.scalar.dma_start(iota8_sb[:], iota8_p[:])
            wt_sb = cpool.tile([128, 128], BF16, tag="wt")
            nc.scalar.dma_start(wt_sb[:], wt_p[:])
            nidx_regs = [nc.gpsimd.alloc_register(f"nidx{i}") for i in range(8)]

            # zero every gather buffer once, FIRST on the DVE queue (these
            # gate the first gathers via buffer reuse): trimmed pad lanes
            # leave stale SBUF in gt rows, and uninitialized bf16 can be
            # NaN/Inf -- 0 * NaN would poison the PSUM accumulation.
            for _ in range(16):
                gz = gpool.tile([128, GB * 128], BF16, tag="gt")
                nc.vector.memset(gz[:], 0.0)

            sf = cpool.tile([128, TC8], BF16, tag="sf")
            nc.vector.tensor_copy(sf[:], si[:])

            # dis_dst = 1/sqrt(deg+1) from rowptr diffs, [128, WPC] f32
            r0f = cpool.tile([128, WPC], F32, tag="r0f")
            nc.vector.tensor_copy(r0f[:], r0i[:])
            r1f = cpool.tile([128, WPC], F32, tag="r1f")
            nc.vector.tensor_copy(r1f[:], r1i[:])
            dg = cpool.tile([128, WPC], F32, tag="dg")
            nc.vector.tensor_tensor(out=dg[:], in0=r1f[:], in1=r0f[:], op=OP.subtract)
            nc.vector.tensor_scalar_add(out=dg[:], in0=dg[:], scalar1=1.0)
            rc = cpool.tile([128, WPC], F32, tag="rc")
            nc.vector.reciprocal(rc[:], dg[:])
            dis_s = cpool.tile([128, WPC], F32, tag="dis")
            nc.scalar.activation(dis_s[:], rc[:], AF.Sqrt)

            # remaining gidx parts (sync queue), xst parts (vector queue)
            for i in range(1, len(gparts)):
                t, c0 = gidx_sb[i]
                nc.sync.dma_start(t[:], gidx_p[:, c0 * 8 : gparts[i][3] * 8])
            xst_sb = cpool.tile([128, NPAD], BF16, tag="xst")
            xw = (WPC + XST_PARTS - 1) // XST_PARTS  # windows per xst part
            for i in range(XST_PARTS):
                a, b = i * xw * 128, min((i + 1) * xw * 128, NPAD)
                nc.scalar.dma_start(xst_sb[:, a:b], xst_p[:, a:b])

            tables = [x_p[0:SPLIT, :], x_p[SPLIT:NROWS, :]]
            out_v = out_p[:].rearrange("(u p) d -> p u d", p=128)

            # prefetch the first 8 per-call valid counts into the rotating
            # register bank; the loop keeps loading 8 calls ahead so a
            # gather's decode never stalls on its count register.
            for i in range(min(8, ncalls)):
                nc.gpsimd.reg_load(nidx_regs[i % 8], nidx_sb[0:1, i : i + 1])

            # --- S group builder: chunk-major S[p, k*128 + c] built per
            # SG-chunk group with two plain DVE passes (eq then scale).
            # Chunk-major keeps the matmul rhs contiguous; the broadcast
            # operands run at 1x DVE rate but per-group ops amortize well.
            sgroups = {}

            def build_sgroup(gb):
                Sw = spool.tile([128, 128 * SG], BF16, tag="S")
                sw = Sw[:]
                o = gb * SG
                dims = [sw.ap[0], [128, SG], [1, 128]]  # (k, c) iteration
                outap = bass.AP(sw.tensor, sw.offset, dims)
                in0 = bass.AP(sf.tensor, sf.offset + o, [sf.ap[0], [1, SG], [0, 128]])
                ii = iota8_sb[:]
                in1 = bass.AP(ii.tensor, ii.offset, [ii.ap[0], [128, SG], [1, 128]])
                nc.vector.tensor_tensor(out=outap, in0=in0, in1=in1, op=OP.is_equal)
                in2 = bass.AP(scf.tensor, scf.offset + o, [scf.ap[0], [1, SG], [0, 128]])
                nc.vector.tensor_tensor(out=outap, in0=outap, in1=in2, op=OP.mult)
                sgroups[gb] = Sw
                return Sw

            qrr = 0
            for w in range(WPC):
                nmm_w = int(K[0, w] + K[1, w])
                mm_w = 0
                ps = psA.tile([128, 128], F32, tag="pacc")
                for p in range(2):
                    Kw = int(K[p, w])
                    c0 = int(cbase[p, w])
                    done = 0
                    while done < Kw:
                        nch = min(GB, Kw - done)
                        cc = c0 + done
                        pi = part_of_w[w]
                        ptile, pbase = gidx_sb[pi]
                        lofs = (cc - pbase) * 8
                        gt = gpool.tile([128, GB * 128], BF16, tag="gt")
                        gv = gt[:, : nch * 128].rearrange(
                            "p (b e) -> p b e", e=128
                        )
                        nc.gpsimd.dma_gather(
                            gv,
                            tables[p],
                            ptile[:, lofs : lofs + nch * 8],
                            nch * 128,
                            nidx_regs[qrr % 8],
                            D,
                            queue_num=qrr % NQ,
                        )
                        if qrr + 8 < ncalls:
                            nc.gpsimd.reg_load(
                                nidx_regs[(qrr + 8) % 8],
                                nidx_sb[0:1, qrr + 8 : qrr + 9],
                            )
                        qrr += 1
                        for k in range(nch):
                            g = cc + k
                            gb, kk = divmod(g, SG)
                            Sw = sgroups.get(gb)
                            if Sw is None:
                                Sw = build_sgroup(gb)
                            nc.tensor.matmul(
                                ps[:],
                                lhsT=gt[:, k * 128 : (k + 1) * 128],
                                rhs=Sw[:, kk * 128 : (kk + 1) * 128],
                                start=(mm_w == 0),
                                stop=(mm_w == nmm_w - 1),
                            )
                            mm_w += 1
                        done += nch

                # --- finalize window w
                wsl = slice(w * 128, (w + 1) * 128)
                att = fpool.tile([128, 128], BF16, tag="att")
                if nmm_w:
                    nc.vector.tensor_tensor(
                        out=att[:], in0=ps[:], in1=xst_sb[:, wsl], op=OP.add
                    )
                else:
                    nc.vector.tensor_copy(att[:], xst_sb[:, wsl])
                po = psO.tile([128, 128], F32, tag="po")
                nc.tensor.matmul(
                    po[:], lhsT=att[:], rhs=wt_sb[:], start=True, stop=True
                )
                ot = fpool.tile([128, 128], BF16, tag="ot")
                nc.scalar.activation(
                    ot[:], po[:], AF.Relu, scale=dis_s[:, w : w + 1]
                )
                nc.sync.dma_start(out_v[:, w, :], ot[:])

    nc.compile()
    return nc


# ---------------------------------------------------------------- runner
_CACHE = {}


def _get_program(K):
    key = K.tobytes()
    if key not in _CACHE:
        _CACHE[key] = build_program(K)
    return _CACHE[key]


def make_in_maps(x, W, prep):
    x = np.asarray(x, np.float32)
    xb = np.zeros((NROWS, D), ml_dtypes.bfloat16)
    xb[:N] = x.astype(ml_dtypes.bfloat16)
    Wt = np.ascontiguousarray(np.asarray(W, np.float32).T).astype(
        ml_dtypes.bfloat16
    )
    # iota8[p, k*128 + c] = c  (chunk-major)
    iota8 = np.tile(
        np.tile(np.arange(128, dtype=np.float32), SG)[None, :], (128, 1)
    ).astype(ml_dtypes.bfloat16)
    in_maps = []
    for c in range(C):
        cd = prep["cores"][c]
        xst = np.zeros((D, NPAD), ml_dtypes.bfloat16)
        xst[:, :NPC] = x[c * NPC : (c + 1) * NPC].T.astype(ml_dtypes.bfloat16)
        in_maps.append(
            {
                "xb": xb,
                "xst": xst,
                "wt": Wt,
                "iota8": iota8,
                "rp0s": cd["rp0s"],
                "rp1s": cd["rp1s"],
                "gidx": cd["gidx"],
                "slots": cd["slots"],
                "scales": cd["scales"],
                "nidx": cd["nidx"],
            }
        )
    return in_maps


def run_spmd(x, edge_index, W, trace=False, **spmd_kwargs):
    prep = host_prep(edge_index)
    nc = _get_program(prep["K"])
    in_maps = make_in_maps(x, W, prep)
    res = run_bass_kernel_spmd(nc, in_maps, list(range(C)), trace=trace, **spmd_kwargs)
    out = np.concatenate(
        [np.asarray(res.results[c]["out"][:NPC], np.float32) for c in range(C)],
        axis=0,
    )
    return out, res


def kernel(x, edge_index, N=None, W=None, **_):
    out, _res = run_spmd(np.asarray(x), np.asarray(edge_index), np.asarray(W))
    return out


# revision 40
# speedup vs baseline: 1.0242x; 1.0131x over previous
"""GCN layer (message passing + linear + ReLU) on 8 Trainium2 NeuronCores.

out = relu(((scatter_add(h[src] -> dst) + x) * dis) @ W.T),
h = x * dis,  dis = rsqrt(deg + 1),  deg = in-degree via dst counts.

Strategy (SPMD, one program on 8 cores):
  - Nodes sharded contiguously: core c owns rows [c*6250, (c+1)*6250).
  - Host partitions edges by dst owner and sorts by dst (index-only work);
    degree reaches the device as CSR rowptr slices, so deg = rowptr diff
    and dis = 1/sqrt(deg+1) are computed on device in f32.
  - No h table: each edge's h[src] = x[src]*dis[src] is obtained by
    gathering x[src] (bf16, 256B rows) straight from a replicated bf16
    copy of x, with dis[src] folded into the one-hot scatter matrix S
    (S values are host-precomputed per-edge scales -- graph metadata).
  - Scatter-add on-chip: edges sorted by dst fall into windows of 128
    owned nodes; per 128-edge chunk S[e, slot] = dis[src]*(slot==dst-base)
    is built on DVE and the PE accumulates gt.T @ S = agg.T [feat, slot]
    into the window's PSUM bank.
  - S is built in fixed groups of 8 chunks, stored column-major
    (S[p, col*8 + chunk]) so every DVE operand has a packed last dim and
    the 16-bit 2x DVE mode engages; the matmul rhs reads its chunk with a
    stride-8 AP (free for PE: partition dim is the parallel dim).
  - int16 gather indices can't span 50k rows, so each window's edges are
    processed against table halves x[:32768] / x[32768:] (pass lo/hi).
  - Gather calls round-robin over 4 SWDGE queues (each queue is served
    by a different pair of Q7 cores, so descriptor generation overlaps).
  - gidx/xst uploads are split so the first gather/finalize only waits
    for a small first segment.
  - Finalize per window, fused right after its last matmul: att =
    (psum + xT) in bf16, po = att.T @ W.T via PE (no transpose needed:
    agg is feature-major), out = relu(po * dis_dst) via ACT per-partition
    scale, then DMA out.
Chunk counts per (pass, window) are maxed over cores so the single SPMD
program fits every core; shorter cores pad with slot=255 / idx=0 chunks.
"""
import numpy as np
import ml_dtypes

from concourse import bacc, bass, mybir, tile
from concourse.bass_utils import run_bass_kernel_spmd

F32 = mybir.dt.float32
BF16 = mybir.dt.bfloat16
I32 = mybir.dt.int32
I16 = mybir.dt.int16
AF = mybir.ActivationFunctionType
OP = mybir.AluOpType

N = 50000
E = 600000
D = 128
C = 8                      # cores
NPC = N // C               # 6250 nodes per core
WPC = (NPC + 127) // 128   # 49 windows per core
NPAD = WPC * 128           # 6272 padded shard rows
NT_G = (N + 127) // 128    # 391 global node tiles
NROWS = NT_G * 128         # 50048 padded table rows
SPLIT = 32768              # src table split for int16 gather indices
PASS_BOUNDS = [(0, SPLIT), (SPLIT, N)]
GB = 8                     # max chunks per dma_gather call (1024 idxs; >1024
                           # descriptors per SWDGE call crashes the device)
SG = 8                     # chunks per S-group build
NQ = 4                     # SWDGE queues used round-robin
GIDX_SPLITS = [0, 4, 16, 27, 38, WPC]  # gidx upload part boundaries (windows)
XST_PARTS = 7              # xst upload split (7 windows each)


def _chunk_layout(K):
    """Global chunk index base per (pass, window), window-major interleaved."""
    K = np.asarray(K)
    cbase = np.zeros((2, WPC), np.int64)
    cb = 0
    for w in range(WPC):
        for p in range(2):
            cbase[p, w] = cb
            cb += K[p, w]
    return cbase, int(cb)


# ---------------------------------------------------------------- host prep
def host_prep(edge_index):
    src = np.asarray(edge_index[0], dtype=np.int64)
    dst = np.asarray(edge_index[1], dtype=np.int64)
    order = np.argsort(dst, kind="stable")
    ss_all = src[order]
    dd_all = dst[order]
    counts = np.bincount(dst, minlength=N)
    rowptr = np.zeros(N + 1, np.int64)
    rowptr[1:] = np.cumsum(counts)
    dis = 1.0 / np.sqrt(counts.astype(np.float64) + 1.0)  # rsqrt(deg+1)

    per_core = []
    need = np.zeros((C, 2, WPC), np.int64)
    for c in range(C):
        e0, e1 = rowptr[c * NPC], rowptr[(c + 1) * NPC]
        ss, dd = ss_all[e0:e1], dd_all[e0:e1]
        per_core.append((ss, dd))
        for p, (lo, hi) in enumerate(PASS_BOUNDS):
            m = (ss >= lo) & (ss < hi)
            w = (dd[m] - c * NPC) // 128
            need[c, p] = np.bincount(w, minlength=WPC)
    K = np.ceil(need.max(axis=0) / 128).astype(np.int64)  # [2, WPC]

    cbase, TC = _chunk_layout(K)
    TC8 = ((TC + SG - 1) // SG) * SG

    cores = []
    for c in range(C):
        ss, dd = per_core[c]
        g = np.zeros(TC * 128, np.int64)
        s = np.full(TC8 * 128, 255, np.int64)
        sc = np.zeros(TC8 * 128, np.float64)
        for p, (lo, hi) in enumerate(PASS_BOUNDS):
            m = (ss >= lo) & (ss < hi)
            sg = ss[m]                       # global src id
            sp = sg - lo                     # index into table half
            dloc = dd[m] - c * NPC
            w = dloc // 128
            cnt = np.bincount(w, minlength=WPC)
            ofs = np.zeros(WPC, np.int64)
            ofs[1:] = np.cumsum(cnt[:-1])
            pos = cbase[p, w] * 128 + (np.arange(len(sp)) - ofs[w])
            g[pos] = sp
            s[pos] = dloc - w * 128
            sc[pos] = dis[sg]
        d = {}
        # pad gather slots use idx 0 (a real row, masked by S=0). Trailing
        # -1 trimming + per-call count registers was tried and measured
        # SLOWER: the register read stalls the Pool decode pipeline by more
        # than the ~13% descriptor saving (and mismatched counts desync the
        # SWDGE ring and wedge the device).
        # gather idx layout [128, TC*8]: stream pos j at [j%16, j//16],
        # replicated across the 8 groups of 16 partitions.
        d["gidx"] = np.tile(g.reshape(-1, 16).T.astype(np.int16), (8, 1)).copy()
        # slot / scale layout [128, TC8]: stream pos j at [j%128, j//128]
        d["slots"] = s.reshape(-1, 128).T.astype(np.int16).copy()
        d["scales"] = sc.reshape(-1, 128).T.astype(ml_dtypes.bfloat16).copy()
        n0 = c * NPC
        rpv = np.full(NPAD + 1, rowptr[min((c + 1) * NPC, N)], np.int64)
        rpv[: NPC + 1] = rowptr[n0 : n0 + NPC + 1]
        d["rp0s"] = rpv[:NPAD].reshape(WPC, 128).T.astype(np.int32).copy()
        d["rp1s"] = rpv[1 : NPAD + 1].reshape(WPC, 128).T.astype(np.int32).copy()
        cores.append(d)
    return dict(K=K, cbase=cbase, TC=TC, TC8=TC8, cores=cores)


def _gidx_parts(K, cbase, TC):
    """Split windows into ranges per GIDX_SPLITS; return per-part
    (w0, w1, chunk_start, chunk_end) so gather calls address their tile."""
    parts = []
    for i in range(len(GIDX_SPLITS) - 1):
        w0, w1 = GIDX_SPLITS[i], GIDX_SPLITS[i + 1]
        c0 = int(cbase[0, w0])
        c1 = int(cbase[0, w1]) if w1 < WPC else TC
        parts.append((w0, w1, c0, c1))
    return parts


# ---------------------------------------------------------------- program
def build_program(K):
    K = np.asarray(K)
    cbase, TC = _chunk_layout(K)
    TC8 = ((TC + SG - 1) // SG) * SG
    NSG = TC // SG + (1 if TC % SG else 0)  # S groups actually consumed

    nc = bacc.Bacc(
        None, target_bir_lowering=False, debug=False, num_swdge_queues=NQ
    )

    x_p = nc.dram_tensor("xb", [NROWS, D], BF16, kind="ExternalInput")
    xst_p = nc.dram_tensor("xst", [D, NPAD], BF16, kind="ExternalInput")
    wt_p = nc.dram_tensor("wt", [D, D], BF16, kind="ExternalInput")
    iota8_p = nc.dram_tensor("iota8", [128, 128 * SG], BF16, kind="ExternalInput")
    rp0s_p = nc.dram_tensor("rp0s", [128, WPC], I32, kind="ExternalInput")
    rp1s_p = nc.dram_tensor("rp1s", [128, WPC], I32, kind="ExternalInput")
    gidx_p = nc.dram_tensor("gidx", [128, TC * 8], I16, kind="ExternalInput")
    slots_p = nc.dram_tensor("slots", [128, TC8], I16, kind="ExternalInput")
    scales_p = nc.dram_tensor("scales", [128, TC8], BF16, kind="ExternalInput")
    out_p = nc.dram_tensor("out", [NPAD, D], BF16, kind="ExternalOutput")

    gparts = _gidx_parts(K, cbase, TC)

    with tile.TileContext(nc) as tc:
        with (
            tc.tile_pool(name="const", bufs=1) as cpool,
            tc.tile_pool(name="gather", bufs=16) as gpool,
            tc.tile_pool(name="sel", bufs=8) as spool,
            tc.tile_pool(name="fin", bufs=3) as fpool,
            tc.tile_pool(name="psA", bufs=4, space="PSUM") as psA,
            tc.tile_pool(name="psO", bufs=2, space="PSUM") as psO,
        ):
            # --- uploads; gidx parts on the sync queue (first part small so
            # gathers start early), metadata on the scalar queue, xst on the
            # vector queue -- three independent HWDGE rings.
            gidx_sb = {}
            part_of_w = {}
            for i, (w0, w1, c0, c1) in enumerate(gparts):
                gt_ = cpool.tile([128, (c1 - c0) * 8], I16, tag=f"gidx{i}")
                gidx_sb[i] = (gt_, c0)
                for w in range(w0, w1):
                    part_of_w[w] = i
            nc.sync.dma_start(
                gidx_sb[0][0][:], gidx_p[:, gparts[0][2] * 8 : gparts[0][3] * 8]
            )
            r0i = cpool.tile([128, WPC], I32, tag="r0i")
            nc.scalar.dma_start(r0i[:], rp0s_p[:])
            r1i = cpool.tile([128, WPC], I32, tag="r1i")
            nc.scalar.dma_start(r1i[:], rp1s_p[:])
            si = cpool.tile([128, TC8], I16, tag="si")
            nc.scalar.dma_start(si[:], slots_p[:])
            scf = cpool.tile([128, TC8], BF16, tag="scf")
            nc.scalar.dma_start(scf[:], scales_p[:])
            iota8_sb = cpool.tile([128, 128 * SG], BF16, tag="iota8")
            nc.scalar.dma_start(iota8_sb[:], iota8_p[:])
            wt_sb = cpool.tile([128, 128], BF16, tag="wt")
            nc.scalar.dma_start(wt_sb[:], wt_p[:])
            sf = cpool.tile([128, TC8], BF16, tag="sf")
            nc.vector.tensor_copy(sf[:], si[:])

            # dis_dst = 1/sqrt(deg+1) from rowptr diffs, [128, WPC] f32
            r0f = cpool.tile([128, WPC], F32, tag="r0f")
            nc.vector.tensor_copy(r0f[:], r0i[:])
            r1f = cpool.tile([128, WPC], F32, tag="r1f")
            nc.vector.tensor_copy(r1f[:], r1i[:])
            dg = cpool.tile([128, WPC], F32, tag="dg")
            nc.vector.tensor_tensor(out=dg[:], in0=r1f[:], in1=r0f[:], op=OP.subtract)
            nc.vector.tensor_scalar_add(out=dg[:], in0=dg[:], scalar1=1.0)
            rc = cpool.tile([128, WPC], F32, tag="rc")
            nc.vector.reciprocal(rc[:], dg[:])
            dis_s = cpool.tile([128, WPC], F32, tag="dis")
            nc.scalar.activation(dis_s[:], rc[:], AF.Sqrt)

            # remaining gidx parts (sync queue), xst parts (vector queue)
            for i in range(1, len(gparts)):
                t, c0 = gidx_sb[i]
                nc.sync.dma_start(t[:], gidx_p[:, c0 * 8 : gparts[i][3] * 8])
            xst_sb = cpool.tile([128, NPAD], BF16, tag="xst")
            xw = (WPC + XST_PARTS - 1) // XST_PARTS  # windows per xst part
            for i in range(XST_PARTS):
                a, b = i * xw * 128, min((i + 1) * xw * 128, NPAD)
                nc.scalar.dma_start(xst_sb[:, a:b], xst_p[:, a:b])

            tables = [x_p[0:SPLIT, :], x_p[SPLIT:NROWS, :]]
            out_v = out_p[:].rearrange("(u p) d -> p u d", p=128)

            # --- S group builder: chunk-major S[p, k*128 + c] built per
            # SG-chunk group with two plain DVE passes (eq then scale).
            # Chunk-major keeps the matmul rhs contiguous; the broadcast
            # operands run at 1x DVE rate but per-group ops amortize well.
            sgroups = {}

            def build_sgroup(gb):
                Sw = spool.tile([128, 128 * SG], BF16, tag="S")
                sw = Sw[:]
                o = gb * SG
                dims = [sw.ap[0], [128, SG], [1, 128]]  # (k, c) iteration
                outap = bass.AP(sw.tensor, sw.offset, dims)
                in0 = bass.AP(sf.tensor, sf.offset + o, [sf.ap[0], [1, SG], [0, 128]])
                ii = iota8_sb[:]
                in1 = bass.AP(ii.tensor, ii.offset, [ii.ap[0], [128, SG], [1, 128]])
                nc.vector.tensor_tensor(out=outap, in0=in0, in1=in1, op=OP.is_equal)
                in2 = bass.AP(scf.tensor, scf.offset + o, [scf.ap[0], [1, SG], [0, 128]])
                nc.vector.tensor_tensor(out=outap, in0=outap, in1=in2, op=OP.mult)
                sgroups[gb] = Sw
                return Sw

            qrr = 0
            for w in range(WPC):
                nmm_w = int(K[0, w] + K[1, w])
                mm_w = 0
                ps = psA.tile([128, 128], F32, tag="pacc")
                for p in range(2):
                    Kw = int(K[p, w])
                    c0 = int(cbase[p, w])
                    done = 0
                    while done < Kw:
                        nch = min(GB, Kw - done)
                        cc = c0 + done
                        pi = part_of_w[w]
                        ptile, pbase = gidx_sb[pi]
                        lofs = (cc - pbase) * 8
                        gt = gpool.tile([128, GB * 128], BF16, tag="gt")
                        gv = gt[:, : nch * 128].rearrange(
                            "p (b e) -> p b e", e=128
                        )
                        nc.gpsimd.dma_gather(
                            gv,
                            tables[p],
                            ptile[:, lofs : lofs + nch * 8],
                            nch * 128,
                            nch * 128,
                            D,
                            queue_num=qrr % NQ,
                        )
                        qrr += 1
                        for k in range(nch):
                            g = cc + k
                            gb, kk = divmod(g, SG)
                            Sw = sgroups.get(gb)
                            if Sw is None:
                                Sw = build_sgroup(gb)
                            nc.tensor.matmul(
                                ps[:],
                                lhsT=gt[:, k * 128 : (k + 1) * 128],
                                rhs=Sw[:, kk * 128 : (kk + 1) * 128],
                                start=(mm_w == 0),
                                stop=(mm_w == nmm_w - 1),
                            )
                            mm_w += 1
                        done += nch

                # --- finalize window w
                wsl = slice(w * 128, (w + 1) * 128)
                att = fpool.tile([128, 128], BF16, tag="att")
                if nmm_w:
                    nc.vector.tensor_tensor(
                        out=att[:], in0=ps[:], in1=xst_sb[:, wsl], op=OP.add
                    )
                else:
                    nc.vector.tensor_copy(att[:], xst_sb[:, wsl])
                po = psO.tile([128, 128], F32, tag="po")
                nc.tensor.matmul(
                    po[:], lhsT=att[:], rhs=wt_sb[:], start=True, stop=True
                )
                ot = fpool.tile([128, 128], BF16, tag="ot")
                nc.scalar.activation(
                    ot[:], po[:], AF.Relu, scale=dis_s[:, w : w + 1]
                )
                nc.sync.dma_start(out_v[:, w, :], ot[:])

    nc.compile()
    return nc


# ---------------------------------------------------------------- runner
_CACHE = {}


def _get_program(K):
    key = K.tobytes()
    if key not in _CACHE:
        _CACHE[key] = build_program(K)
    return _CACHE[key]


def make_in_maps(x, W, prep):
    x = np.asarray(x, np.float32)
    xb = np.zeros((NROWS, D), ml_dtypes.bfloat16)
    xb[:N] = x.astype(ml_dtypes.bfloat16)
    Wt = np.ascontiguousarray(np.asarray(W, np.float32).T).astype(
        ml_dtypes.bfloat16
    )
    # iota8[p, k*128 + c] = c  (chunk-major)
    iota8 = np.tile(
        np.tile(np.arange(128, dtype=np.float32), SG)[None, :], (128, 1)
    ).astype(ml_dtypes.bfloat16)
    in_maps = []
    for c in range(C):
        cd = prep["cores"][c]
        xst = np.zeros((D, NPAD), ml_dtypes.bfloat16)
        xst[:, :NPC] = x[c * NPC : (c + 1) * NPC].T.astype(ml_dtypes.bfloat16)
        in_maps.append(
            {
                "xb": xb,
                "xst": xst,
                "wt": Wt,
                "iota8": iota8,
                "rp0s": cd["rp0s"],
                "rp1s": cd["rp1s"],
                "gidx": cd["gidx"],
                "slots": cd["slots"],
                "scales": cd["scales"],
            }
        )
    return in_maps


def run_spmd(x, edge_index, W, trace=False, **spmd_kwargs):
    prep = host_prep(edge_index)
    nc = _get_program(prep["K"])
    in_maps = make_in_maps(x, W, prep)
    res = run_bass_kernel_spmd(nc, in_maps, list(range(C)), trace=trace, **spmd_kwargs)
    out = np.concatenate(
        [np.asarray(res.results[c]["out"][:NPC], np.float32) for c in range(C)],
        axis=0,
    )
    return out, res


def kernel(x, edge_index, N=None, W=None, **_):
    out, _res = run_spmd(np.asarray(x), np.asarray(edge_index), np.asarray(W))
    return out


# revision 48
# speedup vs baseline: 1.1040x; 1.0779x over previous
"""GCN layer (message passing + linear + ReLU) on 8 Trainium2 NeuronCores.

out = relu(((scatter_add(h[src] -> dst) + x) * dis) @ W.T),
h = x * dis,  dis = rsqrt(deg + 1),  deg = in-degree via dst counts.

Strategy (SPMD, one program on 8 cores):
  - Nodes sharded contiguously: core c owns rows [c*6250, (c+1)*6250).
  - Host partitions edges by dst owner and sorts by dst (index-only work);
    degree reaches the device as CSR rowptr slices, so deg = rowptr diff
    and dis = 1/sqrt(deg+1) are computed on device in f32.
  - No h table: each edge's h[src] = x[src]*dis[src] is obtained by
    gathering x[src] (bf16, 256B rows) straight from a replicated bf16
    copy of x, with dis[src] folded into the one-hot scatter matrix S
    (S values are host-precomputed per-edge scales -- graph metadata).
  - Scatter-add on-chip: edges sorted by dst fall into windows of 128
    owned nodes; per 128-edge chunk S[e, slot] = dis[src]*(slot==dst-base)
    is built on DVE and the PE accumulates gt.T @ S = agg.T [feat, slot]
    into the window's PSUM bank.
  - S is built in fixed groups of 8 chunks on DVE (is_equal vs an iota
    table, then multiply by the per-edge scale), stored chunk-major so
    every matmul rhs slice is contiguous (strided rhs measured +90ns per
    matmul; col-major S with 2x DVE mode traded DVE time for PE time at
    no net win since the kernel is gather-dispatch-bound).
  - int16 gather indices can't span 50k rows, so each window's edges are
    processed against table halves x[:32768] / x[32768:] (pass lo/hi).
  - Gather calls round-robin over 4 SWDGE queues (each queue is served
    by a different pair of Q7 cores, so descriptor generation overlaps).
  - gidx/xst uploads are split so the first gather/finalize only waits
    for a small first segment.
  - Finalize per window, fused right after its last matmul: att =
    (psum + xT) in bf16, po = att.T @ W.T via PE (no transpose needed:
    agg is feature-major), out = relu(po * dis_dst) via ACT per-partition
    scale, then DMA out.
Chunk counts per (pass, window) are maxed over cores so the single SPMD
program fits every core; shorter cores pad with slot=255 / idx=0 chunks.
"""
import numpy as np
import ml_dtypes

from concourse import bacc, bass, mybir, tile
from concourse.bass_utils import run_bass_kernel_spmd

F32 = mybir.dt.float32
BF16 = mybir.dt.bfloat16
I32 = mybir.dt.int32
I16 = mybir.dt.int16
AF = mybir.ActivationFunctionType
OP = mybir.AluOpType

N = 50000
E = 600000
D = 128
C = 8                      # cores
NPC = N // C               # 6250 nodes per core
WPC = (NPC + 127) // 128   # 49 windows per core
NPAD = WPC * 128           # 6272 padded shard rows
NT_G = (N + 127) // 128    # 391 global node tiles
NROWS = NT_G * 128         # 50048 padded table rows
SPLIT = 32768              # src table split for int16 gather indices
PASS_BOUNDS = [(0, SPLIT), (SPLIT, N)]
GB = 8                     # max chunks per dma_gather call (1024 idxs; >1024
                           # descriptors per SWDGE call crashes the device)
SG = 8                     # chunks per S-group build
NQ = 4                     # SWDGE queues used round-robin
GIDX_SPLITS = [0, 4, 16, 27, 38, WPC]  # gidx upload part boundaries (windows)
XST_PARTS = 7              # xst upload split (7 windows each)


def _chunk_layout(K):
    """Global chunk index base per (pass, window), window-major interleaved."""
    K = np.asarray(K)
    cbase = np.zeros((2, WPC), np.int64)
    cb = 0
    for w in range(WPC):
        for p in range(2):
            cbase[p, w] = cb
            cb += K[p, w]
    return cbase, int(cb)


# ---------------------------------------------------------------- host prep
def host_prep(edge_index):
    src = np.asarray(edge_index[0], dtype=np.int64)
    dst = np.asarray(edge_index[1], dtype=np.int64)
    order = np.argsort(dst, kind="stable")
    ss_all = src[order]
    dd_all = dst[order]
    counts = np.bincount(dst, minlength=N)
    rowptr = np.zeros(N + 1, np.int64)
    rowptr[1:] = np.cumsum(counts)
    dis = 1.0 / np.sqrt(counts.astype(np.float64) + 1.0)  # rsqrt(deg+1)

    per_core = []
    need = np.zeros((C, 2, WPC), np.int64)
    for c in range(C):
        e0, e1 = rowptr[c * NPC], rowptr[(c + 1) * NPC]
        ss, dd = ss_all[e0:e1], dd_all[e0:e1]
        per_core.append((ss, dd))
        for p, (lo, hi) in enumerate(PASS_BOUNDS):
            m = (ss >= lo) & (ss < hi)
            w = (dd[m] - c * NPC) // 128
            need[c, p] = np.bincount(w, minlength=WPC)
    # Each core processes ITS windows sorted by edge count (descending).
    # Aligning the order statistics across cores shrinks the max-over-core
    # chunk counts K, and the final processed window is everyone's
    # smallest (shorter tail after the last gather).
    perm = np.argsort(-need.sum(axis=1), axis=1, kind="stable")  # [C, WPC]
    need_s = np.take_along_axis(need, perm[:, None, :], axis=2)
    K = np.ceil(need_s.max(axis=0) / 128).astype(np.int64)  # [2, WPC]

    cbase, TC = _chunk_layout(K)
    TC8 = ((TC + SG - 1) // SG) * SG

    cores = []
    for c in range(C):
        ss, dd = per_core[c]
        slot_of_w = np.empty(WPC, np.int64)
        slot_of_w[perm[c]] = np.arange(WPC)
        g = np.zeros(TC * 128, np.int64)
        s = np.full(TC8 * 128, 255, np.int64)
        sc = np.zeros(TC8 * 128, np.float64)
        for p, (lo, hi) in enumerate(PASS_BOUNDS):
            m = (ss >= lo) & (ss < hi)
            sg = ss[m]                       # global src id
            sp = sg - lo                     # index into table half
            dloc = dd[m] - c * NPC
            w = dloc // 128
            cnt = np.bincount(w, minlength=WPC)
            ofs = np.zeros(WPC, np.int64)
            ofs[1:] = np.cumsum(cnt[:-1])
            pos = cbase[p, slot_of_w[w]] * 128 + (np.arange(len(sp)) - ofs[w])
            g[pos] = sp
            s[pos] = dloc - w * 128
            sc[pos] = dis[sg]
        d = {}
        # pad gather slots use idx 0 (a real row, masked by S=0). Trailing
        # -1 trimming + per-call count registers was tried and measured
        # SLOWER: the register read stalls the Pool decode pipeline by more
        # than the ~13% descriptor saving (and mismatched counts desync the
        # SWDGE ring and wedge the device).
        # gather idx layout [128, TC*8]: stream pos j at [j%16, j//16],
        # replicated across the 8 groups of 16 partitions.
        d["gidx"] = np.tile(g.reshape(-1, 16).T.astype(np.int16), (8, 1)).copy()
        # slot / scale layout [128, TC8]: stream pos j at [j%128, j//128]
        d["slots"] = s.reshape(-1, 128).T.astype(np.int16).copy()
        d["scales"] = sc.reshape(-1, 128).T.astype(ml_dtypes.bfloat16).copy()
        n0 = c * NPC
        rpv = np.full(NPAD + 1, rowptr[min((c + 1) * NPC, N)], np.int64)
        rpv[: NPC + 1] = rowptr[n0 : n0 + NPC + 1]
        rp0m = rpv[:NPAD].reshape(WPC, 128)[perm[c]]
        rp1m = rpv[1 : NPAD + 1].reshape(WPC, 128)[perm[c]]
        d["rp0s"] = rp0m.T.astype(np.int32).copy()
        d["rp1s"] = rp1m.T.astype(np.int32).copy()
        d["perm"] = perm[c]
        cores.append(d)
    return dict(K=K, cbase=cbase, TC=TC, TC8=TC8, cores=cores)


def _gidx_parts(K, cbase, TC):
    """Split windows into ranges per GIDX_SPLITS; return per-part
    (w0, w1, chunk_start, chunk_end) so gather calls address their tile."""
    parts = []
    for i in range(len(GIDX_SPLITS) - 1):
        w0, w1 = GIDX_SPLITS[i], GIDX_SPLITS[i + 1]
        c0 = int(cbase[0, w0])
        c1 = int(cbase[0, w1]) if w1 < WPC else TC
        parts.append((w0, w1, c0, c1))
    return parts


# ---------------------------------------------------------------- program
def build_program(K):
    K = np.asarray(K)
    cbase, TC = _chunk_layout(K)
    TC8 = ((TC + SG - 1) // SG) * SG
    NSG = TC // SG + (1 if TC % SG else 0)  # S groups actually consumed

    nc = bacc.Bacc(
        None, target_bir_lowering=False, debug=False, num_swdge_queues=NQ
    )

    x_p = nc.dram_tensor("xb", [NROWS, D], BF16, kind="ExternalInput")
    xst_p = nc.dram_tensor("xst", [D, NPAD], BF16, kind="ExternalInput")
    wt_p = nc.dram_tensor("wt", [D, D], BF16, kind="ExternalInput")
    iota8_p = nc.dram_tensor("iota8", [128, 128 * SG], BF16, kind="ExternalInput")
    rp0s_p = nc.dram_tensor("rp0s", [128, WPC], I32, kind="ExternalInput")
    rp1s_p = nc.dram_tensor("rp1s", [128, WPC], I32, kind="ExternalInput")
    gidx_p = nc.dram_tensor("gidx", [128, TC * 8], I16, kind="ExternalInput")
    slots_p = nc.dram_tensor("slots", [128, TC8], I16, kind="ExternalInput")
    scales_p = nc.dram_tensor("scales", [128, TC8], BF16, kind="ExternalInput")
    out_p = nc.dram_tensor("out", [NPAD, D], BF16, kind="ExternalOutput")

    gparts = _gidx_parts(K, cbase, TC)

    with tile.TileContext(nc) as tc:
        with (
            tc.tile_pool(name="const", bufs=1) as cpool,
            tc.tile_pool(name="gather", bufs=16) as gpool,
            tc.tile_pool(name="sel", bufs=8) as spool,
            tc.tile_pool(name="fin", bufs=3) as fpool,
            tc.tile_pool(name="psA", bufs=4, space="PSUM") as psA,
            tc.tile_pool(name="psO", bufs=2, space="PSUM") as psO,
        ):
            # --- uploads; gidx parts on the sync queue (first part small so
            # gathers start early), metadata on the scalar queue, xst on the
            # vector queue -- three independent HWDGE rings.
            gidx_sb = {}
            part_of_w = {}
            for i, (w0, w1, c0, c1) in enumerate(gparts):
                gt_ = cpool.tile([128, (c1 - c0) * 8], I16, tag=f"gidx{i}")
                gidx_sb[i] = (gt_, c0)
                for w in range(w0, w1):
                    part_of_w[w] = i
            nc.sync.dma_start(
                gidx_sb[0][0][:], gidx_p[:, gparts[0][2] * 8 : gparts[0][3] * 8]
            )
            r0i = cpool.tile([128, WPC], I32, tag="r0i")
            nc.scalar.dma_start(r0i[:], rp0s_p[:])
            r1i = cpool.tile([128, WPC], I32, tag="r1i")
            nc.scalar.dma_start(r1i[:], rp1s_p[:])
            si = cpool.tile([128, TC8], I16, tag="si")
            nc.scalar.dma_start(si[:], slots_p[:])
            scf = cpool.tile([128, TC8], BF16, tag="scf")
            nc.scalar.dma_start(scf[:], scales_p[:])
            iota8_sb = cpool.tile([128, 128 * SG], BF16, tag="iota8")
            nc.scalar.dma_start(iota8_sb[:], iota8_p[:])
            wt_sb = cpool.tile([128, 128], BF16, tag="wt")
            nc.scalar.dma_start(wt_sb[:], wt_p[:])
            sf = cpool.tile([128, TC8], BF16, tag="sf")
            nc.vector.tensor_copy(sf[:], si[:])

            # dis_dst = 1/sqrt(deg+1) from rowptr diffs, [128, WPC] f32
            r0f = cpool.tile([128, WPC], F32, tag="r0f")
            nc.vector.tensor_copy(r0f[:], r0i[:])
            r1f = cpool.tile([128, WPC], F32, tag="r1f")
            nc.vector.tensor_copy(r1f[:], r1i[:])
            dg = cpool.tile([128, WPC], F32, tag="dg")
            nc.vector.tensor_tensor(out=dg[:], in0=r1f[:], in1=r0f[:], op=OP.subtract)
            nc.vector.tensor_scalar_add(out=dg[:], in0=dg[:], scalar1=1.0)
            rc = cpool.tile([128, WPC], F32, tag="rc")
            nc.vector.reciprocal(rc[:], dg[:])
            dis_s = cpool.tile([128, WPC], F32, tag="dis")
            nc.scalar.activation(dis_s[:], rc[:], AF.Sqrt)

            # remaining gidx parts (sync queue), xst parts (vector queue)
            for i in range(1, len(gparts)):
                t, c0 = gidx_sb[i]
                nc.sync.dma_start(t[:], gidx_p[:, c0 * 8 : gparts[i][3] * 8])
            xst_sb = cpool.tile([128, NPAD], BF16, tag="xst")
            xw = (WPC + XST_PARTS - 1) // XST_PARTS  # windows per xst part
            for i in range(XST_PARTS):
                a, b = i * xw * 128, min((i + 1) * xw * 128, NPAD)
                nc.scalar.dma_start(xst_sb[:, a:b], xst_p[:, a:b])

            tables = [x_p[0:SPLIT, :], x_p[SPLIT:NROWS, :]]
            out_v = out_p[:].rearrange("(u p) d -> p u d", p=128)

            # --- S group builder: chunk-major S[p, k*128 + c] built per
            # SG-chunk group with two plain DVE passes (eq then scale).
            # Chunk-major keeps the matmul rhs contiguous; the broadcast
            # operands run at 1x DVE rate but per-group ops amortize well.
            sgroups = {}

            def build_sgroup(gb):
                Sw = spool.tile([128, 128 * SG], BF16, tag="S")
                sw = Sw[:]
                o = gb * SG
                dims = [sw.ap[0], [128, SG], [1, 128]]  # (k, c) iteration
                outap = bass.AP(sw.tensor, sw.offset, dims)
                in0 = bass.AP(sf.tensor, sf.offset + o, [sf.ap[0], [1, SG], [0, 128]])
                ii = iota8_sb[:]
                in1 = bass.AP(ii.tensor, ii.offset, [ii.ap[0], [128, SG], [1, 128]])
                nc.vector.tensor_tensor(out=outap, in0=in0, in1=in1, op=OP.is_equal)
                in2 = bass.AP(scf.tensor, scf.offset + o, [scf.ap[0], [1, SG], [0, 128]])
                nc.vector.tensor_tensor(out=outap, in0=outap, in1=in2, op=OP.mult)
                sgroups[gb] = Sw
                return Sw

            qrr = 0
            for w in range(WPC):
                nmm_w = int(K[0, w] + K[1, w])
                mm_w = 0
                ps = psA.tile([128, 128], F32, tag="pacc")
                for p in range(2):
                    Kw = int(K[p, w])
                    c0 = int(cbase[p, w])
                    done = 0
                    while done < Kw:
                        nch = min(GB, Kw - done)
                        cc = c0 + done
                        pi = part_of_w[w]
                        ptile, pbase = gidx_sb[pi]
                        lofs = (cc - pbase) * 8
                        gt = gpool.tile([128, GB * 128], BF16, tag="gt")
                        gv = gt[:, : nch * 128].rearrange(
                            "p (b e) -> p b e", e=128
                        )
                        nc.gpsimd.dma_gather(
                            gv,
                            tables[p],
                            ptile[:, lofs : lofs + nch * 8],
                            nch * 128,
                            nch * 128,
                            D,
                            queue_num=qrr % NQ,
                        )
                        qrr += 1
                        for k in range(nch):
                            g = cc + k
                            gb, kk = divmod(g, SG)
                            Sw = sgroups.get(gb)
                            if Sw is None:
                                Sw = build_sgroup(gb)
                            nc.tensor.matmul(
                                ps[:],
                                lhsT=gt[:, k * 128 : (k + 1) * 128],
                                rhs=Sw[:, kk * 128 : (kk + 1) * 128],
                                start=(mm_w == 0),
                                stop=(mm_w == nmm_w - 1),
                            )
                            mm_w += 1
                        done += nch

                # --- finalize window w
                wsl = slice(w * 128, (w + 1) * 128)
                att = fpool.tile([128, 128], BF16, tag="att")
                if nmm_w:
                    nc.vector.tensor_tensor(
                        out=att[:], in0=ps[:], in1=xst_sb[:, wsl], op=OP.add
                    )
                else:
                    nc.vector.tensor_copy(att[:], xst_sb[:, wsl])
                po = psO.tile([128, 128], F32, tag="po")
                nc.tensor.matmul(
                    po[:], lhsT=att[:], rhs=wt_sb[:], start=True, stop=True
                )
                ot = fpool.tile([128, 128], BF16, tag="ot")
                nc.scalar.activation(
                    ot[:], po[:], AF.Relu, scale=dis_s[:, w : w + 1]
                )
                nc.sync.dma_start(out_v[:, w, :], ot[:])

    nc.compile()
    return nc


# ---------------------------------------------------------------- runner
_CACHE = {}


def _get_program(K):
    key = K.tobytes()
    if key not in _CACHE:
        _CACHE[key] = build_program(K)
    return _CACHE[key]


def make_in_maps(x, W, prep):
    x = np.asarray(x, np.float32)
    xb = np.zeros((NROWS, D), ml_dtypes.bfloat16)
    xb[:N] = x.astype(ml_dtypes.bfloat16)
    Wt = np.ascontiguousarray(np.asarray(W, np.float32).T).astype(
        ml_dtypes.bfloat16
    )
    # iota8[p, k*128 + c] = c  (chunk-major)
    iota8 = np.tile(
        np.tile(np.arange(128, dtype=np.float32), SG)[None, :], (128, 1)
    ).astype(ml_dtypes.bfloat16)
    in_maps = []
    for c in range(C):
        cd = prep["cores"][c]
        xst = np.zeros((D, NPAD), ml_dtypes.bfloat16)
        xst[:, :NPC] = x[c * NPC : (c + 1) * NPC].T.astype(ml_dtypes.bfloat16)
        # permute window blocks of columns to the processing order
        xst = (
            xst.reshape(D, WPC, 128)[:, cd["perm"], :].reshape(D, NPAD).copy()
        )
        in_maps.append(
            {
                "xb": xb,
                "xst": xst,
                "wt": Wt,
                "iota8": iota8,
                "rp0s": cd["rp0s"],
                "rp1s": cd["rp1s"],
                "gidx": cd["gidx"],
                "slots": cd["slots"],
                "scales": cd["scales"],
            }
        )
    return in_maps


def run_spmd(x, edge_index, W, trace=False, **spmd_kwargs):
    prep = host_prep(edge_index)
    nc = _get_program(prep["K"])
    in_maps = make_in_maps(x, W, prep)
    res = run_bass_kernel_spmd(nc, in_maps, list(range(C)), trace=trace, **spmd_kwargs)
    parts = []
    for c in range(C):
        ob = np.asarray(res.results[c]["out"], np.float32).reshape(WPC, 128, D)
        inv = np.empty(WPC, np.int64)
        inv[prep["cores"][c]["perm"]] = np.arange(WPC)
        parts.append(ob[inv].reshape(NPAD, D)[:NPC])
    return np.concatenate(parts, axis=0), res


def kernel(x, edge_index, N=None, W=None, **_):
    out, _res = run_spmd(np.asarray(x), np.asarray(edge_index), np.asarray(W))
    return out


# revision 49
# speedup vs baseline: 1.1118x; 1.0071x over previous
"""GCN layer (message passing + linear + ReLU) on 8 Trainium2 NeuronCores.

out = relu(((scatter_add(h[src] -> dst) + x) * dis) @ W.T),
h = x * dis,  dis = rsqrt(deg + 1),  deg = in-degree via dst counts.

Strategy (SPMD, one program on 8 cores):
  - Nodes sharded contiguously: core c owns rows [c*6250, (c+1)*6250).
  - Host partitions edges by dst owner and sorts by dst (index-only work);
    degree reaches the device as CSR rowptr slices, so deg = rowptr diff
    and dis = 1/sqrt(deg+1) are computed on device in f32.
  - No h table: each edge's h[src] = x[src]*dis[src] is obtained by
    gathering x[src] (bf16, 256B rows) straight from a replicated bf16
    copy of x, with dis[src] folded into the one-hot scatter matrix S
    (S values are host-precomputed per-edge scales -- graph metadata).
  - Scatter-add on-chip: edges sorted by dst fall into windows of 128
    owned nodes; per 128-edge chunk S[e, slot] = dis[src]*(slot==dst-base)
    is built on DVE and the PE accumulates gt.T @ S = agg.T [feat, slot]
    into the window's PSUM bank.
  - S is built in fixed groups of 8 chunks on DVE (is_equal vs an iota
    table, then multiply by the per-edge scale), stored chunk-major so
    every matmul rhs slice is contiguous (strided rhs measured +90ns per
    matmul; col-major S with 2x DVE mode traded DVE time for PE time at
    no net win since the kernel is gather-dispatch-bound).
  - int16 gather indices can't span 50k rows, so each window's edges are
    processed against table halves x[:32768] / x[32768:] (pass lo/hi).
  - Gather calls round-robin over 4 SWDGE queues (each queue is served
    by a different pair of Q7 cores, so descriptor generation overlaps).
  - gidx/xst uploads are split so the first gather/finalize only waits
    for a small first segment.
  - Finalize per window, fused right after its last matmul: att =
    (psum + xT) in bf16, po = att.T @ W.T via PE (no transpose needed:
    agg is feature-major), out = relu(po * dis_dst) via ACT per-partition
    scale, then DMA out.
Chunk counts per (pass, window) are maxed over cores so the single SPMD
program fits every core; shorter cores pad with slot=255 / idx=0 chunks.
"""
import numpy as np
import ml_dtypes

from concourse import bacc, bass, mybir, tile
from concourse.bass_utils import run_bass_kernel_spmd

F32 = mybir.dt.float32
BF16 = mybir.dt.bfloat16
I32 = mybir.dt.int32
I16 = mybir.dt.int16
AF = mybir.ActivationFunctionType
OP = mybir.AluOpType

N = 50000
E = 600000
D = 128
C = 8                      # cores
NPC = N // C               # 6250 nodes per core
WPC = (NPC + 127) // 128   # 49 windows per core
NPAD = WPC * 128           # 6272 padded shard rows
NT_G = (N + 127) // 128    # 391 global node tiles
NROWS = NT_G * 128         # 50048 padded table rows
SPLIT = 32768              # src table split for int16 gather indices
PASS_BOUNDS = [(0, SPLIT), (SPLIT, N)]
GB = 8                     # max chunks per dma_gather call (1024 idxs; >1024
                           # descriptors per SWDGE call crashes the device)
SG = 8                     # chunks per S-group build
NQ = 4                     # SWDGE queues used round-robin
GIDX_SPLITS = [0, 1, 3, 8, 20, WPC]  # gidx upload part boundaries (windows);
                                     # tiny first parts so the first gathers
                                     # only wait ~1-2us of idx upload
XST_PARTS = 7              # xst upload split (7 windows each)


def _chunk_layout(K):
    """Global chunk index base per (pass, window), window-major interleaved."""
    K = np.asarray(K)
    cbase = np.zeros((2, WPC), np.int64)
    cb = 0
    for w in range(WPC):
        for p in range(2):
            cbase[p, w] = cb
            cb += K[p, w]
    return cbase, int(cb)


# ---------------------------------------------------------------- host prep
def host_prep(edge_index):
    src = np.asarray(edge_index[0], dtype=np.int64)
    dst = np.asarray(edge_index[1], dtype=np.int64)
    order = np.argsort(dst, kind="stable")
    ss_all = src[order]
    dd_all = dst[order]
    counts = np.bincount(dst, minlength=N)
    rowptr = np.zeros(N + 1, np.int64)
    rowptr[1:] = np.cumsum(counts)
    dis = 1.0 / np.sqrt(counts.astype(np.float64) + 1.0)  # rsqrt(deg+1)

    per_core = []
    need = np.zeros((C, 2, WPC), np.int64)
    for c in range(C):
        e0, e1 = rowptr[c * NPC], rowptr[(c + 1) * NPC]
        ss, dd = ss_all[e0:e1], dd_all[e0:e1]
        per_core.append((ss, dd))
        for p, (lo, hi) in enumerate(PASS_BOUNDS):
            m = (ss >= lo) & (ss < hi)
            w = (dd[m] - c * NPC) // 128
            need[c, p] = np.bincount(w, minlength=WPC)
    # Each core processes ITS windows sorted by edge count (descending).
    # Aligning the order statistics across cores shrinks the max-over-core
    # chunk counts K, and the final processed window is everyone's
    # smallest (shorter tail after the last gather).
    perm = np.argsort(-need.sum(axis=1), axis=1, kind="stable")  # [C, WPC]
    need_s = np.take_along_axis(need, perm[:, None, :], axis=2)
    K = np.ceil(need_s.max(axis=0) / 128).astype(np.int64)  # [2, WPC]

    cbase, TC = _chunk_layout(K)
    TC8 = ((TC + SG - 1) // SG) * SG

    cores = []
    for c in range(C):
        ss, dd = per_core[c]
        slot_of_w = np.empty(WPC, np.int64)
        slot_of_w[perm[c]] = np.arange(WPC)
        g = np.zeros(TC * 128, np.int64)
        s = np.full(TC8 * 128, 255, np.int64)
        sc = np.zeros(TC8 * 128, np.float64)
        for p, (lo, hi) in enumerate(PASS_BOUNDS):
            m = (ss >= lo) & (ss < hi)
            sg = ss[m]                       # global src id
            sp = sg - lo                     # index into table half
            dloc = dd[m] - c * NPC
            w = dloc // 128
            cnt = np.bincount(w, minlength=WPC)
            ofs = np.zeros(WPC, np.int64)
            ofs[1:] = np.cumsum(cnt[:-1])
            pos = cbase[p, slot_of_w[w]] * 128 + (np.arange(len(sp)) - ofs[w])
            g[pos] = sp
            s[pos] = dloc - w * 128
            sc[pos] = dis[sg]
        d = {}
        # pad gather slots use idx 0 (a real row, masked by S=0). Trailing
        # -1 trimming + per-call count registers was tried and measured
        # SLOWER: the register read stalls the Pool decode pipeline by more
        # than the ~13% descriptor saving (and mismatched counts desync the
        # SWDGE ring and wedge the device).
        # gather idx layout [128, TC*8]: stream pos j at [j%16, j//16],
        # replicated across the 8 groups of 16 partitions.
        d["gidx"] = np.tile(g.reshape(-1, 16).T.astype(np.int16), (8, 1)).copy()
        # slot / scale layout [128, TC8]: stream pos j at [j%128, j//128]
        d["slots"] = s.reshape(-1, 128).T.astype(np.int16).copy()
        d["scales"] = sc.reshape(-1, 128).T.astype(ml_dtypes.bfloat16).copy()
        n0 = c * NPC
        rpv = np.full(NPAD + 1, rowptr[min((c + 1) * NPC, N)], np.int64)
        rpv[: NPC + 1] = rowptr[n0 : n0 + NPC + 1]
        rp0m = rpv[:NPAD].reshape(WPC, 128)[perm[c]]
        rp1m = rpv[1 : NPAD + 1].reshape(WPC, 128)[perm[c]]
        d["rp0s"] = rp0m.T.astype(np.int32).copy()
        d["rp1s"] = rp1m.T.astype(np.int32).copy()
        d["perm"] = perm[c]
        cores.append(d)
    return dict(K=K, cbase=cbase, TC=TC, TC8=TC8, cores=cores)


def _gidx_parts(K, cbase, TC):
    """Split windows into ranges per GIDX_SPLITS; return per-part
    (w0, w1, chunk_start, chunk_end) so gather calls address their tile."""
    parts = []
    for i in range(len(GIDX_SPLITS) - 1):
        w0, w1 = GIDX_SPLITS[i], GIDX_SPLITS[i + 1]
        c0 = int(cbase[0, w0])
        c1 = int(cbase[0, w1]) if w1 < WPC else TC
        parts.append((w0, w1, c0, c1))
    return parts


# ---------------------------------------------------------------- program
def build_program(K):
    K = np.asarray(K)
    cbase, TC = _chunk_layout(K)
    TC8 = ((TC + SG - 1) // SG) * SG
    NSG = TC // SG + (1 if TC % SG else 0)  # S groups actually consumed

    nc = bacc.Bacc(
        None, target_bir_lowering=False, debug=False, num_swdge_queues=NQ
    )

    x_p = nc.dram_tensor("xb", [NROWS, D], BF16, kind="ExternalInput")
    xst_p = nc.dram_tensor("xst", [D, NPAD], BF16, kind="ExternalInput")
    wt_p = nc.dram_tensor("wt", [D, D], BF16, kind="ExternalInput")
    iota8_p = nc.dram_tensor("iota8", [128, 128 * SG], BF16, kind="ExternalInput")
    rp0s_p = nc.dram_tensor("rp0s", [128, WPC], I32, kind="ExternalInput")
    rp1s_p = nc.dram_tensor("rp1s", [128, WPC], I32, kind="ExternalInput")
    gidx_p = nc.dram_tensor("gidx", [128, TC * 8], I16, kind="ExternalInput")
    slots_p = nc.dram_tensor("slots", [128, TC8], I16, kind="ExternalInput")
    scales_p = nc.dram_tensor("scales", [128, TC8], BF16, kind="ExternalInput")
    out_p = nc.dram_tensor("out", [NPAD, D], BF16, kind="ExternalOutput")

    gparts = _gidx_parts(K, cbase, TC)

    with tile.TileContext(nc) as tc:
        with (
            tc.tile_pool(name="const", bufs=1) as cpool,
            tc.tile_pool(name="gather", bufs=16) as gpool,
            tc.tile_pool(name="sel", bufs=8) as spool,
            tc.tile_pool(name="fin", bufs=3) as fpool,
            tc.tile_pool(name="psA", bufs=4, space="PSUM") as psA,
            tc.tile_pool(name="psO", bufs=2, space="PSUM") as psO,
        ):
            # --- uploads; gidx parts on the sync queue (first part small so
            # gathers start early), metadata on the scalar queue, xst on the
            # vector queue -- three independent HWDGE rings.
            gidx_sb = {}
            part_of_w = {}
            for i, (w0, w1, c0, c1) in enumerate(gparts):
                gt_ = cpool.tile([128, (c1 - c0) * 8], I16, tag=f"gidx{i}")
                gidx_sb[i] = (gt_, c0)
                for w in range(w0, w1):
                    part_of_w[w] = i
            nc.sync.dma_start(
                gidx_sb[0][0][:], gidx_p[:, gparts[0][2] * 8 : gparts[0][3] * 8]
            )
            r0i = cpool.tile([128, WPC], I32, tag="r0i")
            nc.scalar.dma_start(r0i[:], rp0s_p[:])
            r1i = cpool.tile([128, WPC], I32, tag="r1i")
            nc.scalar.dma_start(r1i[:], rp1s_p[:])
            si = cpool.tile([128, TC8], I16, tag="si")
            nc.scalar.dma_start(si[:], slots_p[:])
            scf = cpool.tile([128, TC8], BF16, tag="scf")
            nc.scalar.dma_start(scf[:], scales_p[:])
            iota8_sb = cpool.tile([128, 128 * SG], BF16, tag="iota8")
            nc.scalar.dma_start(iota8_sb[:], iota8_p[:])
            wt_sb = cpool.tile([128, 128], BF16, tag="wt")
            nc.scalar.dma_start(wt_sb[:], wt_p[:])
            sf = cpool.tile([128, TC8], BF16, tag="sf")
            nc.vector.tensor_copy(sf[:], si[:])

            # dis_dst = 1/sqrt(deg+1) from rowptr diffs, [128, WPC] f32
            r0f = cpool.tile([128, WPC], F32, tag="r0f")
            nc.vector.tensor_copy(r0f[:], r0i[:])
            r1f = cpool.tile([128, WPC], F32, tag="r1f")
            nc.vector.tensor_copy(r1f[:], r1i[:])
            dg = cpool.tile([128, WPC], F32, tag="dg")
            nc.vector.tensor_tensor(out=dg[:], in0=r1f[:], in1=r0f[:], op=OP.subtract)
            nc.vector.tensor_scalar_add(out=dg[:], in0=dg[:], scalar1=1.0)
            rc = cpool.tile([128, WPC], F32, tag="rc")
            nc.vector.reciprocal(rc[:], dg[:])
            dis_s = cpool.tile([128, WPC], F32, tag="dis")
            nc.scalar.activation(dis_s[:], rc[:], AF.Sqrt)

            # remaining gidx parts (sync queue), xst parts (vector queue)
            for i in range(1, len(gparts)):
                t, c0 = gidx_sb[i]
                nc.sync.dma_start(t[:], gidx_p[:, c0 * 8 : gparts[i][3] * 8])
            xst_sb = cpool.tile([128, NPAD], BF16, tag="xst")
            xw = (WPC + XST_PARTS - 1) // XST_PARTS  # windows per xst part
            for i in range(XST_PARTS):
                a, b = i * xw * 128, min((i + 1) * xw * 128, NPAD)
                nc.scalar.dma_start(xst_sb[:, a:b], xst_p[:, a:b])

            tables = [x_p[0:SPLIT, :], x_p[SPLIT:NROWS, :]]
            out_v = out_p[:].rearrange("(u p) d -> p u d", p=128)

            # --- S group builder: chunk-major S[p, k*128 + c] built per
            # SG-chunk group with two plain DVE passes (eq then scale).
            # Chunk-major keeps the matmul rhs contiguous; the broadcast
            # operands run at 1x DVE rate but per-group ops amortize well.
            sgroups = {}

            def build_sgroup(gb):
                Sw = spool.tile([128, 128 * SG], BF16, tag="S")
                sw = Sw[:]
                o = gb * SG
                dims = [sw.ap[0], [128, SG], [1, 128]]  # (k, c) iteration
                outap = bass.AP(sw.tensor, sw.offset, dims)
                in0 = bass.AP(sf.tensor, sf.offset + o, [sf.ap[0], [1, SG], [0, 128]])
                ii = iota8_sb[:]
                in1 = bass.AP(ii.tensor, ii.offset, [ii.ap[0], [128, SG], [1, 128]])
                nc.vector.tensor_tensor(out=outap, in0=in0, in1=in1, op=OP.is_equal)
                in2 = bass.AP(scf.tensor, scf.offset + o, [scf.ap[0], [1, SG], [0, 128]])
                nc.vector.tensor_tensor(out=outap, in0=outap, in1=in2, op=OP.mult)
                sgroups[gb] = Sw
                return Sw

            qrr = 0
            for w in range(WPC):
                nmm_w = int(K[0, w] + K[1, w])
                mm_w = 0
                ps = psA.tile([128, 128], F32, tag="pacc")
                for p in range(2):
                    Kw = int(K[p, w])
                    c0 = int(cbase[p, w])
                    done = 0
                    while done < Kw:
                        nch = min(GB, Kw - done)
                        cc = c0 + done
                        pi = part_of_w[w]
                        ptile, pbase = gidx_sb[pi]
                        lofs = (cc - pbase) * 8
                        gt = gpool.tile([128, GB * 128], BF16, tag="gt")
                        gv = gt[:, : nch * 128].rearrange(
                            "p (b e) -> p b e", e=128
                        )
                        nc.gpsimd.dma_gather(
                            gv,
                            tables[p],
                            ptile[:, lofs : lofs + nch * 8],
                            nch * 128,
                            nch * 128,
                            D,
                            queue_num=qrr % NQ,
                        )
                        qrr += 1
                        for k in range(nch):
                            g = cc + k
                            gb, kk = divmod(g, SG)
                            Sw = sgroups.get(gb)
                            if Sw is None:
                                Sw = build_sgroup(gb)
                            nc.tensor.matmul(
                                ps[:],
                                lhsT=gt[:, k * 128 : (k + 1) * 128],
                                rhs=Sw[:, kk * 128 : (kk + 1) * 128],
                                start=(mm_w == 0),
                                stop=(mm_w == nmm_w - 1),
                            )
                            mm_w += 1
                        done += nch

                # --- finalize window w
                wsl = slice(w * 128, (w + 1) * 128)
                att = fpool.tile([128, 128], BF16, tag="att")
                if nmm_w:
                    nc.vector.tensor_tensor(
                        out=att[:], in0=ps[:], in1=xst_sb[:, wsl], op=OP.add
                    )
                else:
                    nc.vector.tensor_copy(att[:], xst_sb[:, wsl])
                po = psO.tile([128, 128], F32, tag="po")
                nc.tensor.matmul(
                    po[:], lhsT=att[:], rhs=wt_sb[:], start=True, stop=True
                )
                ot = fpool.tile([128, 128], BF16, tag="ot")
                nc.scalar.activation(
                    ot[:], po[:], AF.Relu, scale=dis_s[:, w : w + 1]
                )
                nc.sync.dma_start(out_v[:, w, :], ot[:])

    nc.compile()
    return nc


# ---------------------------------------------------------------- runner
_CACHE = {}


def _get_program(K):
    key = K.tobytes()
    if key not in _CACHE:
        _CACHE[key] = build_program(K)
    return _CACHE[key]


def make_in_maps(x, W, prep):
    x = np.asarray(x, np.float32)
    xb = np.zeros((NROWS, D), ml_dtypes.bfloat16)
    xb[:N] = x.astype(ml_dtypes.bfloat16)
    Wt = np.ascontiguousarray(np.asarray(W, np.float32).T).astype(
        ml_dtypes.bfloat16
    )
    # iota8[p, k*128 + c] = c  (chunk-major)
    iota8 = np.tile(
        np.tile(np.arange(128, dtype=np.float32), SG)[None, :], (128, 1)
    ).astype(ml_dtypes.bfloat16)
    in_maps = []
    for c in range(C):
        cd = prep["cores"][c]
        xst = np.zeros((D, NPAD), ml_dtypes.bfloat16)
        xst[:, :NPC] = x[c * NPC : (c + 1) * NPC].T.astype(ml_dtypes.bfloat16)
        # permute window blocks of columns to the processing order
        xst = (
            xst.reshape(D, WPC, 128)[:, cd["perm"], :].reshape(D, NPAD).copy()
        )
        in_maps.append(
            {
                "xb": xb,
                "xst": xst,
                "wt": Wt,
                "iota8": iota8,
                "rp0s": cd["rp0s"],
                "rp1s": cd["rp1s"],
                "gidx": cd["gidx"],
                "slots": cd["slots"],
                "scales": cd["scales"],
            }
        )
    return in_maps


def run_spmd(x, edge_index, W, trace=False, **spmd_kwargs):
    prep = host_prep(edge_index)
    nc = _get_program(prep["K"])
    in_maps = make_in_maps(x, W, prep)
    res = run_bass_kernel_spmd(nc, in_maps, list(range(C)), trace=trace, **spmd_kwargs)
    parts = []
    for c in range(C):
        ob = np.asarray(res.results[c]["out"], np.float32).reshape(WPC, 128, D)
        inv = np.empty(WPC, np.int64)
        inv[prep["cores"][c]["perm"]] = np.arange(WPC)
        parts.append(ob[inv].reshape(NPAD, D)[:NPC])
    return np.concatenate(parts, axis=0), res


def kernel(x, edge_index, N=None, W=None, **_):
    out, _res = run_spmd(np.asarray(x), np.asarray(edge_index), np.asarray(W))
    return out
